# revision 20
# baseline (speedup 1.0000x reference)
"""GQA attention kernel for Trainium2, 8 NeuronCores.

Sharding: core i handles (batch b = i//4, kv-group g = i%4) -> 4 query heads.
Each core computes its group's partial out-projection; host sums the 4
partials per batch element (the "all-reduce after out_proj").

v3 dataflow (trace-driven, see v2 notes in git history):
  - Block-major pipeline: each 512-row block streams its x chunks in,
    projects K/V/Q, then runs its attention rows. The next block's x is
    prefetched before the current block's attention so the tensor engine
    never stalls at block boundaries (keeps the HAM clock-gate warm).
  - No Sqrt / DVE-reciprocal anywhere: rsqrt(x) = exp(-0.5*ln(x)) and
    1/x = exp(-ln(x)) on the scalar engine; one pre-loaded activation
    table set (natural_log_exp) serves the whole kernel.
  - Attention processes heads in pairs: one [128, 2, 512] PSUM score
    tile, one [128, 1024] exp per k-tile. Softmax denominators come from
    ones-matmuls over the bf16 exp accumulator, with the final k-tile
    accumulated directly in PSUM so no DVE op sits on the PE critical
    path at pair boundaries.
  - Out-projection tiles of finished blocks are interleaved as PE filler
    into later Q-projection/attention streams.
  - Weights arrive as one large contiguous DMA each (host pre-arranges
    partition-major layouts); x blocks in 512 KiB pieces.
"""

import sys

sys.path.insert(0, "/opt/trn_rl_repo")

import numpy as np
import ml_dtypes

import concourse.bass as bass
import concourse.tile as tile
from concourse import bacc
from concourse import mybir
from concourse.bass import ts
from concourse.bass_utils import run_bass_kernel_spmd

BF16 = ml_dtypes.bfloat16

B = 2           # batch
S = 2048        # sequence
D = 2048        # model dim
HL = 4          # heads per core (local)
DH = 128        # head dim
NC_ = 16        # d-chunks of 128
NT = 16         # seq tiles of 128
NB = 4          # seq blocks of 512
BLK = 512
EPS = 1e-6

_CACHED_NC = None


def build_nc():
    dt = mybir.dt
    nc = bacc.Bacc()

    xT = nc.declare_dram_parameter("xT", [NB, 128, NC_ * BLK], dt.bfloat16, isOutput=False)
    wq = nc.declare_dram_parameter("wq", [128, NC_ * HL * DH], dt.bfloat16, isOutput=False)
    wk = nc.declare_dram_parameter("wk", [128, NC_ * DH], dt.bfloat16, isOutput=False)
    wv = nc.declare_dram_parameter("wv", [128, NC_ * DH], dt.bfloat16, isOutput=False)
    wo = nc.declare_dram_parameter("wo", [128, HL * D], dt.bfloat16, isOutput=False)
    cosq = nc.declare_dram_parameter("cosq", [128, S], dt.bfloat16, isOutput=False)
    sinq = nc.declare_dram_parameter("sinq", [128, S], dt.bfloat16, isOutput=False)
    cosk = nc.declare_dram_parameter("cosk", [128, S], dt.bfloat16, isOutput=False)
    sink = nc.declare_dram_parameter("sink", [128, S], dt.bfloat16, isOutput=False)
    tri2 = nc.declare_dram_parameter("tri2", [128, 2, 128], dt.float32, isOutput=False)
    rt = nc.declare_dram_parameter("rt", [128, 128], dt.bfloat16, isOutput=False)
    eye = nc.declare_dram_parameter("eye", [128, 128], dt.bfloat16, isOutput=False)
    ones = nc.declare_dram_parameter("ones", [128, 1], dt.bfloat16, isOutput=False)
    out = nc.declare_dram_parameter("out", [S, D], dt.float32, isOutput=True)

    with tile.TileContext(nc) as tc:
        with (
            tc.tile_pool(name="singles", bufs=1) as singles,
            tc.tile_pool(name="xpool", bufs=2) as xpool,
            tc.tile_pool(name="work", bufs=2) as work,
            tc.tile_pool(name="expp", bufs=4) as expp,
            tc.tile_pool(name="accp", bufs=3) as accp,
            tc.tile_pool(name="rows", bufs=1) as rows,
            tc.tile_pool(name="craw", bufs=6) as craw,
            tc.tile_pool(name="outp", bufs=4) as outp,
            tc.tile_pool(name="ctxp", bufs=2) as ctxp,
            tc.tile_pool(name="pS", bufs=2, space="PSUM") as pS,
            tc.tile_pool(name="pC", bufs=2, space="PSUM") as pC,
            tc.tile_pool(name="pR", bufs=1, space="PSUM") as pR,
        ):
            # ---- resident SBUF tensors ----
            wq_s = singles.tile([128, NC_, HL * DH], dt.bfloat16)
            wk_s = singles.tile([128, NC_, DH], dt.bfloat16)
            wv_s = singles.tile([128, NC_, DH], dt.bfloat16)
            wo_s = singles.tile([128, HL, D], dt.bfloat16)
            cosq_s = singles.tile([128, S], dt.bfloat16)
            sinq_s = singles.tile([128, S], dt.bfloat16)
            cosk_s = singles.tile([128, S], dt.bfloat16)
            sink_s = singles.tile([128, S], dt.bfloat16)
            tri2_s = singles.tile([128, 2, 128], dt.float32)
            rt_s = singles.tile([128, 128], dt.bfloat16)
            eye_s = singles.tile([128, 128], dt.bfloat16)
            ones_s = singles.tile([128, 1], dt.bfloat16)
            rk_s = singles.tile([128, NT], dt.float32)
            eps_s = singles.tile([128, 1], dt.float32)
            nc.vector.memset(eps_s, EPS)
            qT_s = singles.tile([128, HL, S], dt.bfloat16)
            kT_s = singles.tile([128, S], dt.bfloat16)
            vT_s = singles.tile([128, S], dt.bfloat16)
            v_s = singles.tile([128, NT, DH], dt.bfloat16)

            # ---- input DMAs (weights/tables; xT streams per block) ----
            # gpsimd (SW-DGE) queue, one large transfer each, by first use.
            # halves keep the DMA access patterns 2D: a fully-contiguous
            # [128, N] source collapses to one descriptor whose element
            # count overflows the 16-bit src_num_elem ISA field.
            for half in range(2):
                hc = NC_ // 2
                nc.gpsimd.dma_start(
                    out=wk_s[:, half * hc : (half + 1) * hc, :],
                    in_=wk[:, half * hc * DH : (half + 1) * hc * DH],
                )
            for half in range(2):
                hc = NC_ // 2
                nc.gpsimd.dma_start(
                    out=wv_s[:, half * hc : (half + 1) * hc, :],
                    in_=wv[:, half * hc * DH : (half + 1) * hc * DH],
                )
            for quarter in range(4):
                qc = NC_ // 4
                nc.gpsimd.dma_start(
                    out=wq_s[:, quarter * qc : (quarter + 1) * qc, :],
                    in_=wq[:, quarter * qc * HL * DH : (quarter + 1) * qc * HL * DH],
                )
            nc.gpsimd.dma_start(out=eye_s[:], in_=eye[:])
            nc.gpsimd.dma_start(out=cosk_s[:], in_=cosk[:])
            nc.gpsimd.dma_start(out=sink_s[:], in_=sink[:])
            nc.gpsimd.dma_start(out=rt_s[:], in_=rt[:])
            nc.gpsimd.dma_start(out=ones_s[:], in_=ones[:])
            nc.gpsimd.dma_start(out=tri2_s[:], in_=tri2[:])
            nc.gpsimd.dma_start(out=cosq_s[:], in_=cosq[:])
            nc.gpsimd.dma_start(out=sinq_s[:], in_=sinq[:])
            for half in range(2):
                hh_ = HL // 2
                nc.gpsimd.dma_start(
                    out=wo_s[:, half * hh_ : (half + 1) * hh_, :],
                    in_=wo[:, half * hh_ * D : (half + 1) * hh_ * D],
                )

            Exp = mybir.ActivationFunctionType.Exp
            Ln = mybir.ActivationFunctionType.Ln
            Copy = mybir.ActivationFunctionType.Copy

            # Pre-load the one table set that covers exp+ln+copy; without
            # this the insertion pass alternates exp_and_others <->
            # natural_log (41 loads x ~2.7us of ACT time).
            from concourse.hw_specs import get_activation_tables

            tabs = get_activation_tables(nc.m.arch)
            set_id = list(tabs).index("natural_log_exp_and_others")
            nc.scalar.add_instruction(
                mybir.InstLoadActFuncSet(
                    name=nc.get_next_instruction_name(),
                    act_func_set_id=set_id,
                    ins=[],
                    outs=[],
                )
            )

            def load_xb(Bb):
                xb = xpool.tile([128, NC_, BLK], dt.bfloat16, tag="xblk")
                for g4 in range(4):
                    nc.sync.dma_start(
                        out=xb[:, 4 * g4 : 4 * g4 + 4, :],
                        in_=xT[Bb, :, 4 * g4 * BLK : (4 * g4 + 4) * BLK],
                    )
                return xb

            def rope(dst_ap, qn, cos_s, sin_s, Bb):
                """dst = rope(qn) in bf16; qn is SBUF bf16 [128, 512]."""
                ps_rot = pC.tile([128, BLK], dt.float32, tag="acc")
                nc.tensor.matmul(ps_rot, rt_s, qn, start=True, stop=True)
                t1 = work.tile([128, BLK], dt.bfloat16, tag="t1")
                nc.vector.tensor_mul(t1, ps_rot, sin_s[:, ts(Bb, BLK)])
                t2 = work.tile([128, BLK], dt.bfloat16, tag="t2")
                nc.vector.tensor_mul(t2, qn, cos_s[:, ts(Bb, BLK)])
                nc.vector.tensor_add(dst_ap, t2, t1)

            # out-proj filler machinery: tiles of completed blocks are
            # interleaved into later PE streams to cover ACT-bound spans.
            filler = []

            def emit_filler(pool=None, tag="row", on_act=False):
                if not filler:
                    return
                ctxn_t, stl, st, oc = filler.pop(0)
                ps_o = (pool or pR).tile([128, BLK], dt.float32, tag=tag)
                for h in range(HL):
                    nc.tensor.matmul(
                        ps_o, ctxn_t[:, h, ts(stl, 128)],
                        wo_s[:, h, ts(oc, BLK)],
                        start=(h == 0), stop=(h == HL - 1),
                    )
                osb = outp.tile([128, BLK], dt.float32, tag="osb")
                if on_act:
                    # drain phase only: ACT has no exp stream to feed there
                    nc.scalar.activation(osb, ps_o, Copy)
                else:
                    nc.vector.tensor_copy(osb, ps_o)
                nc.sync.dma_start(out=out[ts(st, 128), ts(oc, BLK)], in_=osb)

            xb = load_xb(0)
            for Bb in range(NB):
                # ---- K + V projections, c-major ----
                ps_k = pC.tile([128, BLK], dt.float32, tag="acc")
                ps_v = pC.tile([128, BLK], dt.float32, tag="acc")
                for c in range(NC_):
                    nc.tensor.matmul(
                        ps_k, wk_s[:, c, :], xb[:, c, :],
                        start=(c == 0), stop=(c == NC_ - 1),
                    )
                    nc.tensor.matmul(
                        ps_v, wv_s[:, c, :], xb[:, c, :],
                        start=(c == 0), stop=(c == NC_ - 1),
                    )
                kraw = work.tile([128, BLK], dt.bfloat16, tag="raw")
                nc.scalar.activation(kraw, ps_k, Copy)
                nc.vector.tensor_copy(vT_s[:, ts(Bb, BLK)], ps_v)
                k2 = work.tile([128, BLK], dt.bfloat16, tag="sq")
                nc.vector.tensor_mul(k2, kraw, kraw)
                ps_rk = pR.tile([128, 4], dt.float32, tag="row")
                for tt in range(4):
                    nc.tensor.matmul(
                        ps_rk[:, tt : tt + 1], k2[:, ts(tt, 128)], ones_s,
                        start=True, stop=True,
                    )
                # rk = exp(-0.5*ln(ms/DH + eps))
                nc.scalar.activation(ps_rk, ps_rk, Ln, scale=1.0 / DH, bias=eps_s)
                nc.scalar.activation(
                    rk_s[:, Bb * 4 : Bb * 4 + 4], ps_rk, Exp, scale=-0.5
                )
                rope(kT_s[:, ts(Bb, BLK)], kraw, cosk_s, sink_s, Bb)

                # V transposes: vT [dh, sk] -> v_s [sk, t, dh]
                for tt in range(4):
                    t = Bb * 4 + tt
                    t_ps = pC.tile([128, DH], dt.bfloat16, tag="acc")
                    nc.tensor.transpose(t_ps, vT_s[:, ts(t, 128)], eye_s)
                    nc.vector.tensor_copy(v_s[:, t, :], t_ps)

                # ---- Q projections for this block, rms via ln/exp ----
                # software-pipelined: matmuls+rowsums for both pairs first,
                # then the rope chains (whose latency hides under them),
                # with out-proj filler between.
                qraws = {}
                rqs = {}
                for hp in range(2):
                    ps_ss = pR.tile([1, 2, BLK], dt.float32, tag="row")
                    for hh in range(2):
                        h = hp * 2 + hh
                        ps_q = pS.tile([128, 2, BLK], dt.float32, tag="big")
                        for c in range(NC_):
                            nc.tensor.matmul(
                                ps_q[:, 0, :], wq_s[:, c, ts(h, DH)],
                                xb[:, c, :],
                                start=(c == 0), stop=(c == NC_ - 1),
                            )
                        qraw = work.tile([128, BLK], dt.bfloat16, tag=f"qr{h}")
                        # DVE, not ACT: in the ACT FIFO this copy queues
                        # behind the previous pair's rq ln/exp (~2.3us) and
                        # stalls the rowsum matmul chain
                        nc.vector.tensor_copy(qraw, ps_q[:, 0, :])
                        qraws[h] = qraw
                        q2 = work.tile([128, BLK], dt.bfloat16, tag="sq")
                        nc.vector.tensor_mul(q2, qraw, qraw)
                        nc.tensor.matmul(
                            ps_ss[:, hh, :], ones_s, q2, start=True, stop=True
                        )
                    # rq = exp(-0.5*ln(ss/DH + eps)), fp32 rows
                    nc.scalar.activation(
                        ps_ss, ps_ss, Ln, scale=1.0 / DH, bias=eps_s[:1]
                    )
                    rq = rows.tile([1, 2, BLK], dt.float32, tag=f"rq{hp}")
                    nc.scalar.activation(rq, ps_ss, Exp, scale=-0.5)
                    rqs[hp] = rq
                    emit_filler(pool=pC, tag="acc")
                for hp in range(2):
                    for hh in range(2):
                        h = hp * 2 + hh
                        rq_b = work.tile([128, BLK], dt.float32, tag="rqb")
                        nc.gpsimd.partition_broadcast(rq_b, rqs[hp][:, hh, :])
                        qn = work.tile([128, BLK], dt.bfloat16, tag="qn")
                        nc.vector.tensor_mul(qn, qraws[h], rq_b)
                        rope(qT_s[:, h, ts(Bb, BLK)], qn, cosq_s, sinq_s, Bb)
                        emit_filler(pool=pR, tag="row")

                # ---- prefetch next block's x while attention runs ----
                if Bb + 1 < NB:
                    xb = load_xb(Bb + 1)

                # ---- attention for this block, head pairs ----
                ctxn_t = ctxp.tile([128, HL, BLK], dt.bfloat16, tag="ctxn")
                ntile = 4 * Bb + 4
                for hp in range(2):
                    h0 = hp * 2
                    ps_ctx = []
                    for _ in range(2):
                        ps_cx = pC.tile([128, BLK], dt.float32, tag="acc")
                        ps_ctx.append(ps_cx)
                    acc = accp.tile([128, 2, BLK], dt.bfloat16, tag="sumacc")
                    expS_last = None
                    for t in range(ntile):
                        j0 = max(0, t * 128 - Bb * BLK)
                        ps_S = pS.tile([128, 2, BLK], dt.float32, tag="big")
                        for hh in range(2):
                            nc.tensor.matmul(
                                ps_S[:, hh, j0:],
                                kT_s[:, ts(t, 128)],
                                qT_s[:, h0 + hh, Bb * BLK + j0 : (Bb + 1) * BLK],
                                start=True, stop=True,
                            )
                        if t * 128 >= Bb * BLK:  # diagonal tile: causal mask
                            nc.vector.tensor_add(
                                ps_S[:, :, j0 : j0 + 128],
                                ps_S[:, :, j0 : j0 + 128],
                                tri2_s,
                            )
                        expS = expp.tile([128, 2, BLK], dt.bfloat16, tag="exp")
                        nc.scalar.activation(
                            expS[:, :, j0:], ps_S[:, :, j0:], Exp,
                            scale=rk_s[:, t : t + 1],
                        )
                        for hh in range(2):
                            nc.tensor.matmul(
                                ps_ctx[hh][:, j0:], v_s[:, t, :],
                                expS[:, hh, j0:],
                                start=(t == 0), stop=(t == ntile - 1),
                            )
                        # accumulate exp sums on DVE, except the last tile
                        # (folded into the denominator matmul below so the
                        # PE never waits on DVE at pair end)
                        if t == 0:
                            nc.vector.tensor_copy(acc, expS)
                        elif t < ntile - 1:
                            nc.vector.tensor_add(
                                acc[:, :, j0:], acc[:, :, j0:], expS[:, :, j0:]
                            )
                        else:
                            expS_last = (expS, j0)
                        emit_filler(pool=pR, tag="row")
                    # pair denominators: d = ones @ acc + ones @ expS_last
                    eL, jL = expS_last
                    ps_row = pR.tile([1, 2, BLK], dt.float32, tag="row")
                    for hh in range(2):
                        nc.tensor.matmul(
                            ps_row[:, hh, :], ones_s, acc[:, hh, :],
                            start=True, stop=False,
                        )
                        nc.tensor.matmul(
                            ps_row[:, hh, jL:], ones_s, eL[:, hh, jL:],
                            start=False, stop=True,
                        )
                    nc.scalar.activation(ps_row, ps_row, Ln)
                    recip = rows.tile([1, 2, BLK], dt.bfloat16, tag="recip")
                    nc.scalar.activation(recip, ps_row, Exp, scale=-1.0)
                    recip_b = craw.tile([128, 2, BLK], dt.bfloat16, tag="rb")
                    nc.gpsimd.partition_broadcast(recip_b, recip[:])
                    for hh in range(2):
                        craw_t = craw.tile([128, BLK], dt.bfloat16, tag="craw")
                        nc.vector.tensor_copy(craw_t, ps_ctx[hh])
                        nc.vector.tensor_mul(
                            ctxn_t[:, h0 + hh, :], craw_t, recip_b[:, hh, :]
                        )
                # queue this block's out-proj tiles as filler
                for stl in range(4):
                    st = Bb * 4 + stl
                    for oc in range(4):
                        filler.append((ctxn_t, stl, st, oc))

            # drain remaining out-proj tiles (last block): rotate across
            # the now-free PSUM pools so the groups pipeline.
            i = 0
            while filler:
                pool, tag = [(pS, "big"), (pC, "acc"), (pR, "row")][i % 3]
                emit_filler(pool=pool, tag=tag, on_act=(i % 2 == 0))
                i += 1
    nc.finalize()
    return nc


def _host_inputs(x, cos, sin, Wq, Wk, Wv, Wo, qn_w, kn_w):
    """Build the 8 per-core input maps (host-side sharding + layout prep)."""
    scale = DH ** -0.5
    qn_rot = np.concatenate([qn_w[64:], qn_w[:64]])
    kn_rot = np.concatenate([kn_w[64:], kn_w[:64]])
    cosq = (cos.T * qn_w[:, None] * scale).astype(BF16)
    sinq = (sin.T * qn_rot[:, None] * scale).astype(BF16)
    cosk = (cos.T * kn_w[:, None]).astype(BF16)
    sink = (sin.T * kn_rot[:, None]).astype(BF16)
    ii = np.arange(128)
    tri = np.where(ii[None, :] < ii[:, None], -1e30, 0.0).astype(np.float32)
    tri2 = np.ascontiguousarray(np.stack([tri, tri], axis=1))  # [128, 2, 128]
    R = np.zeros((128, 128), dtype=np.float32)
    R[np.arange(64), np.arange(64) + 64] = -1.0
    R[np.arange(64, 128), np.arange(64)] = 1.0
    rt = R.T.astype(BF16)
    eye = np.eye(128, dtype=BF16)
    ones = np.ones((128, 1), dtype=BF16)

    def pmajor(w, ncol):
        # [NC_*128, ncol] -> [128, NC_*ncol] with w[c*128+p, j] at [p, c*ncol+j]
        n = w.shape[0] // 128
        return np.ascontiguousarray(
            w.reshape(n, 128, ncol).transpose(1, 0, 2).reshape(128, n * ncol)
        ).astype(BF16)

    in_maps = []
    for core in range(8):
        b, g = core // 4, core % 4
        # [NB, 128, NC_*BLK]: xT[Bb, dc, c*BLK+sb] = x[b][Bb*512+sb, c*128+dc]
        xTb = np.ascontiguousarray(
            x[b].reshape(NB, BLK, NC_, 128).transpose(0, 3, 2, 1).reshape(NB, 128, NC_ * BLK)
        ).astype(BF16)
        in_maps.append({
            "xT": xTb,
            "wq": pmajor(Wq[:, g * 512 : (g + 1) * 512], 512),
            "wk": pmajor(Wk[:, g * 128 : (g + 1) * 128], 128),
            "wv": pmajor(Wv[:, g * 128 : (g + 1) * 128], 128),
            "wo": pmajor(Wo[g * 512 : (g + 1) * 512, :], D),
            "cosq": cosq, "sinq": sinq, "cosk": cosk, "sink": sink,
            "tri2": tri2, "rt": rt, "eye": eye, "ones": ones,
        })
    return in_maps


def kernel(x, mask, cos, sin, Wq, Wk, Wv, Wo, qn_w, kn_w, _trace=False):
    global _CACHED_NC
    x = np.asarray(x, dtype=np.float32)
    cos = np.asarray(cos, dtype=np.float32)
    sin = np.asarray(sin, dtype=np.float32)
    Wq = np.asarray(Wq, dtype=np.float32)
    Wk = np.asarray(Wk, dtype=np.float32)
    Wv = np.asarray(Wv, dtype=np.float32)
    Wo = np.asarray(Wo, dtype=np.float32)
    qn_w = np.asarray(qn_w, dtype=np.float32)
    kn_w = np.asarray(kn_w, dtype=np.float32)

    if _CACHED_NC is None:
        _CACHED_NC = build_nc()
    nc = _CACHED_NC
    in_maps = _host_inputs(x, cos, sin, Wq, Wk, Wv, Wo, qn_w, kn_w)
    res = run_bass_kernel_spmd(nc, in_maps, list(range(8)), trace=_trace)
    out = np.zeros((B, S, D), dtype=np.float32)
    for core in range(8):
        b = core // 4
        out[b] += np.asarray(res.results[core]["out"], dtype=np.float32)
    if _trace:
        return out, res
    return out


# revision 25
# speedup vs baseline: 1.1954x; 1.1954x over previous
"""GQA attention kernel for Trainium2, 8 NeuronCores.

Sharding: core i handles (batch b = i//4, kv-group g = i%4) -> 4 query heads.
Each core computes its group's partial out-projection; host sums the 4
partials per batch element (the "all-reduce after out_proj").

Dataflow (trace-driven rework of the 565us staged baseline; 305us measured):
  - Block-major pipeline: each 512-row block streams its x chunks in,
    projects K/V/Q, then runs its attention rows. The next block's x is
    prefetched before the current block's attention so the tensor engine
    never stalls at block boundaries (keeps the HAM clock-gate warm).
  - No Sqrt / DVE-reciprocal anywhere: rsqrt(x) = exp(-0.5*ln(x)) and
    1/x = exp(-ln(x)) on the scalar engine; one pre-loaded activation
    table set (natural_log_exp) serves the whole kernel.
  - Attention processes heads in pairs: one [128, 2, 512] PSUM score
    tile, one [128, 1024] exp per k-tile. Softmax denominators come from
    ones-matmuls over the bf16 exp accumulator, with the final k-tile
    accumulated directly in PSUM so no DVE op sits on the PE critical
    path at pair boundaries.
  - Out-projection tiles of finished blocks are interleaved as PE filler
    into later Q-projection/attention streams.
  - Weights arrive as one large contiguous DMA each (host pre-arranges
    partition-major layouts); x blocks in 512 KiB pieces.
"""

import sys

sys.path.insert(0, "/opt/trn_rl_repo")

import numpy as np
import ml_dtypes

import concourse.bass as bass
import concourse.tile as tile
from concourse import bacc
from concourse import mybir
from concourse.bass import ts
from concourse.bass_utils import run_bass_kernel_spmd

BF16 = ml_dtypes.bfloat16

B = 2           # batch
S = 2048        # sequence
D = 2048        # model dim
HL = 4          # heads per core (local)
DH = 128        # head dim
NC_ = 16        # d-chunks of 128
NT = 16         # seq tiles of 128
NB = 4          # seq blocks of 512
BLK = 512
EPS = 1e-6

_CACHED_NC = None


def build_nc():
    dt = mybir.dt
    nc = bacc.Bacc()

    xT = nc.declare_dram_parameter("xT", [NB, 128, NC_ * BLK], dt.bfloat16, isOutput=False)
    wq = nc.declare_dram_parameter("wq", [128, NC_ * HL * DH], dt.bfloat16, isOutput=False)
    wk = nc.declare_dram_parameter("wk", [128, NC_ * DH], dt.bfloat16, isOutput=False)
    wv = nc.declare_dram_parameter("wv", [128, NC_ * DH], dt.bfloat16, isOutput=False)
    wo = nc.declare_dram_parameter("wo", [128, HL * D], dt.bfloat16, isOutput=False)
    cosq = nc.declare_dram_parameter("cosq", [128, S], dt.bfloat16, isOutput=False)
    sinq = nc.declare_dram_parameter("sinq", [128, S], dt.bfloat16, isOutput=False)
    cosk = nc.declare_dram_parameter("cosk", [128, S], dt.bfloat16, isOutput=False)
    sink = nc.declare_dram_parameter("sink", [128, S], dt.bfloat16, isOutput=False)
    tri2 = nc.declare_dram_parameter("tri2", [128, 2, 128], dt.float32, isOutput=False)
    rt = nc.declare_dram_parameter("rt", [128, 128], dt.bfloat16, isOutput=False)
    eye = nc.declare_dram_parameter("eye", [128, 128], dt.bfloat16, isOutput=False)
    ones = nc.declare_dram_parameter("ones", [128, 1], dt.bfloat16, isOutput=False)
    out = nc.declare_dram_parameter("out", [S, D], dt.float32, isOutput=True)

    with tile.TileContext(nc) as tc:
        with (
            tc.tile_pool(name="singles", bufs=1) as singles,
            tc.tile_pool(name="xpool", bufs=2) as xpool,
            tc.tile_pool(name="work", bufs=2) as work,
            tc.tile_pool(name="expp", bufs=4) as expp,
            tc.tile_pool(name="accp", bufs=3) as accp,
            tc.tile_pool(name="rows", bufs=2) as rows,
            tc.tile_pool(name="craw", bufs=6) as craw,
            tc.tile_pool(name="outp", bufs=4) as outp,
            tc.tile_pool(name="ctxp", bufs=2) as ctxp,
            tc.tile_pool(name="pS", bufs=2, space="PSUM") as pS,
            tc.tile_pool(name="pC", bufs=2, space="PSUM") as pC,
            tc.tile_pool(name="pR", bufs=1, space="PSUM") as pR,
        ):
            # ---- resident SBUF tensors ----
            wq_s = singles.tile([128, NC_, HL * DH], dt.bfloat16)
            wk_s = singles.tile([128, NC_, DH], dt.bfloat16)
            wv_s = singles.tile([128, NC_, DH], dt.bfloat16)
            wo_s = singles.tile([128, HL, D], dt.bfloat16)
            cosq_s = singles.tile([128, S], dt.bfloat16)
            sinq_s = singles.tile([128, S], dt.bfloat16)
            cosk_s = singles.tile([128, S], dt.bfloat16)
            sink_s = singles.tile([128, S], dt.bfloat16)
            tri2_s = singles.tile([128, 2, 128], dt.float32)
            rt_s = singles.tile([128, 128], dt.bfloat16)
            eye_s = singles.tile([128, 128], dt.bfloat16)
            ones_s = singles.tile([128, 1], dt.bfloat16)
            rk_s = singles.tile([128, NT], dt.float32)
            eps_s = singles.tile([128, 1], dt.float32)
            nc.vector.memset(eps_s, EPS)
            qT_s = singles.tile([128, HL, S], dt.bfloat16)
            kT_s = singles.tile([128, S], dt.bfloat16)
            vT_s = singles.tile([128, S], dt.bfloat16)
            v_s = singles.tile([128, NT, DH], dt.bfloat16)

            # ---- input DMAs (weights/tables; xT streams per block) ----
            # gpsimd (SW-DGE) queue, one large transfer each, by first use.
            # halves keep the DMA access patterns 2D: a fully-contiguous
            # [128, N] source collapses to one descriptor whose element
            # count overflows the 16-bit src_num_elem ISA field.
            for half in range(2):
                hc = NC_ // 2
                nc.gpsimd.dma_start(
                    out=wk_s[:, half * hc : (half + 1) * hc, :],
                    in_=wk[:, half * hc * DH : (half + 1) * hc * DH],
                )
            for half in range(2):
                hc = NC_ // 2
                nc.gpsimd.dma_start(
                    out=wv_s[:, half * hc : (half + 1) * hc, :],
                    in_=wv[:, half * hc * DH : (half + 1) * hc * DH],
                )
            for quarter in range(4):
                qc = NC_ // 4
                nc.gpsimd.dma_start(
                    out=wq_s[:, quarter * qc : (quarter + 1) * qc, :],
                    in_=wq[:, quarter * qc * HL * DH : (quarter + 1) * qc * HL * DH],
                )
            nc.gpsimd.dma_start(out=eye_s[:], in_=eye[:])
            nc.gpsimd.dma_start(out=cosk_s[:], in_=cosk[:])
            nc.gpsimd.dma_start(out=sink_s[:], in_=sink[:])
            nc.gpsimd.dma_start(out=rt_s[:], in_=rt[:])
            nc.gpsimd.dma_start(out=ones_s[:], in_=ones[:])
            nc.gpsimd.dma_start(out=tri2_s[:], in_=tri2[:])
            nc.gpsimd.dma_start(out=cosq_s[:], in_=cosq[:])
            nc.gpsimd.dma_start(out=sinq_s[:], in_=sinq[:])
            for half in range(2):
                hh_ = HL // 2
                nc.gpsimd.dma_start(
                    out=wo_s[:, half * hh_ : (half + 1) * hh_, :],
                    in_=wo[:, half * hh_ * D : (half + 1) * hh_ * D],
                )

            Exp = mybir.ActivationFunctionType.Exp
            Ln = mybir.ActivationFunctionType.Ln
            Copy = mybir.ActivationFunctionType.Copy

            # Pre-load the one table set that covers exp+ln+copy; without
            # this the insertion pass alternates exp_and_others <->
            # natural_log (41 loads x ~2.7us of ACT time).
            from concourse.hw_specs import get_activation_tables

            tabs = get_activation_tables(nc.m.arch)
            set_id = list(tabs).index("natural_log_exp_and_others")
            nc.scalar.add_instruction(
                mybir.InstLoadActFuncSet(
                    name=nc.get_next_instruction_name(),
                    act_func_set_id=set_id,
                    ins=[],
                    outs=[],
                )
            )

            def load_xb(Bb):
                xb = xpool.tile([128, NC_, BLK], dt.bfloat16, tag="xblk")
                # block 0 is on the critical path: finer pieces let the
                # first K matmul start one piece earlier
                step = 2 if Bb == 0 else 4
                for g4 in range(NC_ // step):
                    nc.sync.dma_start(
                        out=xb[:, step * g4 : step * (g4 + 1), :],
                        in_=xT[Bb, :, step * g4 * BLK : step * (g4 + 1) * BLK],
                    )
                return xb

            def rope(dst_ap, qn, cos_s, sin_s, Bb):
                """dst = rope(qn) in bf16; qn is SBUF bf16 [128, 512]."""
                ps_rot = pC.tile([128, BLK], dt.float32, tag="acc")
                nc.tensor.matmul(ps_rot, rt_s, qn, start=True, stop=True)
                t1 = work.tile([128, BLK], dt.bfloat16, tag="t1")
                nc.vector.tensor_mul(t1, ps_rot, sin_s[:, ts(Bb, BLK)])
                t2 = work.tile([128, BLK], dt.bfloat16, tag="t2")
                nc.vector.tensor_mul(t2, qn, cos_s[:, ts(Bb, BLK)])
                nc.vector.tensor_add(dst_ap, t2, t1)

            # out-proj filler machinery: tiles of completed blocks are
            # interleaved into later PE streams to cover ACT-bound spans.
            filler = []

            def emit_filler(pool=None, tag="row", on_act=False):
                if not filler:
                    return
                ctxn_t, stl, st, oc = filler.pop(0)
                ps_o = (pool or pR).tile([128, BLK], dt.float32, tag=tag)
                for h in range(HL):
                    nc.tensor.matmul(
                        ps_o, ctxn_t[:, h, ts(stl, 128)],
                        wo_s[:, h, ts(oc, BLK)],
                        start=(h == 0), stop=(h == HL - 1),
                    )
                osb = outp.tile([128, BLK], dt.float32, tag="osb")
                if on_act:
                    # drain phase only: ACT has no exp stream to feed there
                    nc.scalar.activation(osb, ps_o, Copy)
                else:
                    nc.vector.tensor_copy(osb, ps_o)
                nc.sync.dma_start(out=out[ts(st, 128), ts(oc, BLK)], in_=osb)

            xb = load_xb(0)
            for Bb in range(NB):
                # ---- K + V projections, c-major ----
                ps_k = pC.tile([128, BLK], dt.float32, tag="acc")
                ps_v = pC.tile([128, BLK], dt.float32, tag="acc")
                for c in range(NC_):
                    nc.tensor.matmul(
                        ps_k, wk_s[:, c, :], xb[:, c, :],
                        start=(c == 0), stop=(c == NC_ - 1),
                    )
                    nc.tensor.matmul(
                        ps_v, wv_s[:, c, :], xb[:, c, :],
                        start=(c == 0), stop=(c == NC_ - 1),
                    )
                kraw = work.tile([128, BLK], dt.bfloat16, tag="raw")
                nc.scalar.activation(kraw, ps_k, Copy)
                nc.vector.tensor_copy(vT_s[:, ts(Bb, BLK)], ps_v)
                k2 = work.tile([128, BLK], dt.bfloat16, tag="sq")
                nc.vector.tensor_mul(k2, kraw, kraw)
                ps_rk = pR.tile([128, 4], dt.float32, tag="row")
                for tt in range(4):
                    nc.tensor.matmul(
                        ps_rk[:, tt : tt + 1], k2[:, ts(tt, 128)], ones_s,
                        start=True, stop=True,
                    )
                # rk = exp(-0.5*ln(ms/DH + eps))
                nc.scalar.activation(ps_rk, ps_rk, Ln, scale=1.0 / DH, bias=eps_s)
                nc.scalar.activation(
                    rk_s[:, Bb * 4 : Bb * 4 + 4], ps_rk, Exp, scale=-0.5
                )
                rope(kT_s[:, ts(Bb, BLK)], kraw, cosk_s, sink_s, Bb)

                # V transposes: vT [dh, sk] -> v_s [sk, t, dh]
                for tt in range(4):
                    t = Bb * 4 + tt
                    t_ps = pC.tile([128, DH], dt.bfloat16, tag="acc")
                    nc.tensor.transpose(t_ps, vT_s[:, ts(t, 128)], eye_s)
                    nc.vector.tensor_copy(v_s[:, t, :], t_ps)

                # ---- Q projections for this block, rms via ln/exp ----
                # software-pipelined: matmuls+rowsums for both pairs first,
                # then the rope chains (whose latency hides under them),
                # with out-proj filler between.
                qraws = {}
                rqs = {}
                for hp in range(2):
                    ps_ss = pR.tile([1, 2, BLK], dt.float32, tag="row")
                    for hh in range(2):
                        h = hp * 2 + hh
                        ps_q = pS.tile([128, 2, BLK], dt.float32, tag="big")
                        for c in range(NC_):
                            nc.tensor.matmul(
                                ps_q[:, 0, :], wq_s[:, c, ts(h, DH)],
                                xb[:, c, :],
                                start=(c == 0), stop=(c == NC_ - 1),
                            )
                        qraw = work.tile([128, BLK], dt.bfloat16, tag=f"qr{h}")
                        nc.scalar.activation(qraw, ps_q[:, 0, :], Copy)
                        qraws[h] = qraw
                        q2 = work.tile([128, BLK], dt.bfloat16, tag="sq")
                        nc.vector.tensor_mul(q2, qraw, qraw)
                        nc.tensor.matmul(
                            ps_ss[:, hh, :], ones_s, q2, start=True, stop=True
                        )
                    # rq = exp(-0.5*ln(ss/DH + eps)), fp32 rows
                    nc.scalar.activation(
                        ps_ss, ps_ss, Ln, scale=1.0 / DH, bias=eps_s[:1]
                    )
                    rq = rows.tile([1, 2, BLK], dt.float32, tag=f"rq{hp}")
                    nc.scalar.activation(rq, ps_ss, Exp, scale=-0.5)
                    rqs[hp] = rq
                    emit_filler(pool=pC, tag="acc")
                for hp in range(2):
                    for hh in range(2):
                        h = hp * 2 + hh
                        rq_b = work.tile([128, BLK], dt.float32, tag="rqb")
                        nc.gpsimd.partition_broadcast(rq_b, rqs[hp][:, hh, :])
                        qn = work.tile([128, BLK], dt.bfloat16, tag="qn")
                        nc.vector.tensor_mul(qn, qraws[h], rq_b)
                        rope(qT_s[:, h, ts(Bb, BLK)], qn, cosq_s, sinq_s, Bb)
                        emit_filler(pool=pR, tag="row")

                # ---- prefetch next block's x while attention runs ----
                if Bb + 1 < NB:
                    xb = load_xb(Bb + 1)

                # ---- attention for this block, head pairs ----
                ctxn_t = ctxp.tile([128, HL, BLK], dt.bfloat16, tag="ctxn")
                ntile = 4 * Bb + 4
                for hp in range(2):
                    h0 = hp * 2
                    ps_ctx = []
                    for _ in range(2):
                        ps_cx = pC.tile([128, BLK], dt.float32, tag="acc")
                        ps_ctx.append(ps_cx)
                    acc = accp.tile([128, 2, BLK], dt.bfloat16, tag="sumacc")
                    expS_last = None
                    pending = None
                    for t in range(ntile):
                        j0 = max(0, t * 128 - Bb * BLK)
                        ps_S = pS.tile([128, 2, BLK], dt.float32, tag="big")
                        for hh in range(2):
                            nc.tensor.matmul(
                                ps_S[:, hh, j0:],
                                kT_s[:, ts(t, 128)],
                                qT_s[:, h0 + hh, Bb * BLK + j0 : (Bb + 1) * BLK],
                                start=True, stop=True,
                            )
                        if t * 128 >= Bb * BLK:  # diagonal tile: causal mask
                            nc.vector.tensor_add(
                                ps_S[:, :, j0 : j0 + 128],
                                ps_S[:, :, j0 : j0 + 128],
                                tri2_s,
                            )
                        expS = expp.tile([128, 2, BLK], dt.bfloat16, tag="exp")
                        nc.scalar.activation(
                            expS[:, :, j0:], ps_S[:, :, j0:], Exp,
                            scale=rk_s[:, t : t + 1],
                        )
                        for hh in range(2):
                            nc.tensor.matmul(
                                ps_ctx[hh][:, j0:], v_s[:, t, :],
                                expS[:, hh, j0:],
                                start=(t == 0), stop=(t == ntile - 1),
                            )
                        # accumulate exp sums on DVE, deferred one iteration
                        # so the next diag tile's tri-add doesn't queue
                        # behind this add in the DVE FIFO (which would delay
                        # exp and hold the score slot); the last tile is
                        # folded into the denominator matmul instead.
                        if pending is not None:
                            eP, jP, isF = pending
                            if isF:
                                nc.vector.tensor_copy(acc, eP)
                            else:
                                nc.vector.tensor_add(
                                    acc[:, :, jP:], acc[:, :, jP:],
                                    eP[:, :, jP:],
                                )
                        if t == ntile - 1:
                            expS_last = (expS, j0)
                            pending = None
                        else:
                            pending = (expS, j0, t == 0)
                        emit_filler(pool=pR, tag="row")
                    # pair denominators: d = ones @ acc + ones @ expS_last
                    eL, jL = expS_last
                    ps_row = pR.tile([1, 2, BLK], dt.float32, tag="row")
                    for hh in range(2):
                        nc.tensor.matmul(
                            ps_row[:, hh, :], ones_s, acc[:, hh, :],
                            start=True, stop=False,
                        )
                        nc.tensor.matmul(
                            ps_row[:, hh, jL:], ones_s, eL[:, hh, jL:],
                            start=False, stop=True,
                        )
                    nc.scalar.activation(ps_row, ps_row, Ln)
                    recip = rows.tile([1, 2, BLK], dt.bfloat16, tag="recip")
                    nc.scalar.activation(recip, ps_row, Exp, scale=-1.0)
                    recip_b = craw.tile([128, 2, BLK], dt.bfloat16, tag="rb")
                    nc.gpsimd.partition_broadcast(recip_b, recip[:])
                    for hh in range(2):
                        craw_t = craw.tile([128, BLK], dt.bfloat16, tag="craw")
                        nc.vector.tensor_copy(craw_t, ps_ctx[hh])
                        nc.vector.tensor_mul(
                            ctxn_t[:, h0 + hh, :], craw_t, recip_b[:, hh, :]
                        )
                # queue this block's out-proj tiles as filler
                for stl in range(4):
                    st = Bb * 4 + stl
                    for oc in range(4):
                        filler.append((ctxn_t, stl, st, oc))

            # drain remaining out-proj tiles (last block): rotate across
            # the now-free PSUM pools so the groups pipeline.
            i = 0
            while filler:
                pool, tag = [(pS, "big"), (pC, "acc"), (pR, "row")][i % 3]
                emit_filler(pool=pool, tag=tag, on_act=(i % 2 == 0))
                i += 1
    nc.finalize()
    return nc


def _host_inputs(x, cos, sin, Wq, Wk, Wv, Wo, qn_w, kn_w):
    """Build the 8 per-core input maps (host-side sharding + layout prep)."""
    scale = DH ** -0.5
    qn_rot = np.concatenate([qn_w[64:], qn_w[:64]])
    kn_rot = np.concatenate([kn_w[64:], kn_w[:64]])
    cosq = (cos.T * qn_w[:, None] * scale).astype(BF16)
    sinq = (sin.T * qn_rot[:, None] * scale).astype(BF16)
    cosk = (cos.T * kn_w[:, None]).astype(BF16)
    sink = (sin.T * kn_rot[:, None]).astype(BF16)
    ii = np.arange(128)
    tri = np.where(ii[None, :] < ii[:, None], -1e30, 0.0).astype(np.float32)
    tri2 = np.ascontiguousarray(np.stack([tri, tri], axis=1))  # [128, 2, 128]
    R = np.zeros((128, 128), dtype=np.float32)
    R[np.arange(64), np.arange(64) + 64] = -1.0
    R[np.arange(64, 128), np.arange(64)] = 1.0
    rt = R.T.astype(BF16)
    eye = np.eye(128, dtype=BF16)
    ones = np.ones((128, 1), dtype=BF16)

    def pmajor(w, ncol):
        # [NC_*128, ncol] -> [128, NC_*ncol] with w[c*128+p, j] at [p, c*ncol+j]
        n = w.shape[0] // 128
        return np.ascontiguousarray(
            w.reshape(n, 128, ncol).transpose(1, 0, 2).reshape(128, n * ncol)
        ).astype(BF16)

    in_maps = []
    for core in range(8):
        b, g = core // 4, core % 4
        # [NB, 128, NC_*BLK]: xT[Bb, dc, c*BLK+sb] = x[b][Bb*512+sb, c*128+dc]
        xTb = np.ascontiguousarray(
            x[b].reshape(NB, BLK, NC_, 128).transpose(0, 3, 2, 1).reshape(NB, 128, NC_ * BLK)
        ).astype(BF16)
        in_maps.append({
            "xT": xTb,
            "wq": pmajor(Wq[:, g * 512 : (g + 1) * 512], 512),
            "wk": pmajor(Wk[:, g * 128 : (g + 1) * 128], 128),
            "wv": pmajor(Wv[:, g * 128 : (g + 1) * 128], 128),
            "wo": pmajor(Wo[g * 512 : (g + 1) * 512, :], D),
            "cosq": cosq, "sinq": sinq, "cosk": cosk, "sink": sink,
            "tri2": tri2, "rt": rt, "eye": eye, "ones": ones,
        })
    return in_maps


def kernel(x, mask, cos, sin, Wq, Wk, Wv, Wo, qn_w, kn_w, _trace=False):
    global _CACHED_NC
    x = np.asarray(x, dtype=np.float32)
    cos = np.asarray(cos, dtype=np.float32)
    sin = np.asarray(sin, dtype=np.float32)
    Wq = np.asarray(Wq, dtype=np.float32)
    Wk = np.asarray(Wk, dtype=np.float32)
    Wv = np.asarray(Wv, dtype=np.float32)
    Wo = np.asarray(Wo, dtype=np.float32)
    qn_w = np.asarray(qn_w, dtype=np.float32)
    kn_w = np.asarray(kn_w, dtype=np.float32)

    if _CACHED_NC is None:
        _CACHED_NC = build_nc()
    nc = _CACHED_NC
    in_maps = _host_inputs(x, cos, sin, Wq, Wk, Wv, Wo, qn_w, kn_w)
    res = run_bass_kernel_spmd(nc, in_maps, list(range(8)), trace=_trace)
    out = np.zeros((B, S, D), dtype=np.float32)
    for core in range(8):
        b = core // 4
        out[b] += np.asarray(res.results[core]["out"], dtype=np.float32)
    if _trace:
        return out, res
    return out


# revision 27
# speedup vs baseline: 1.2037x; 1.0069x over previous
"""GQA attention kernel for Trainium2, 8 NeuronCores.

Sharding: core i handles (batch b = i//4, kv-group g = i%4) -> 4 query heads.
Each core computes its group's partial out-projection; host sums the 4
partials per batch element (the "all-reduce after out_proj").

Dataflow (trace-driven rework of the 565us staged baseline; 305us measured):
  - Block-major pipeline: each 512-row block streams its x chunks in,
    projects K/V/Q, then runs its attention rows. The next block's x is
    prefetched before the current block's attention so the tensor engine
    never stalls at block boundaries (keeps the HAM clock-gate warm).
  - No Sqrt / DVE-reciprocal anywhere: rsqrt(x) = exp(-0.5*ln(x)) and
    1/x = exp(-ln(x)) on the scalar engine; one pre-loaded activation
    table set (natural_log_exp) serves the whole kernel.
  - Attention processes heads in pairs: one [128, 2, 512] PSUM score
    tile, one [128, 1024] exp per k-tile. Softmax denominators come from
    ones-matmuls over the bf16 exp accumulator, with the final k-tile
    accumulated directly in PSUM so no DVE op sits on the PE critical
    path at pair boundaries.
  - Out-projection tiles of finished blocks are interleaved as PE filler
    into later Q-projection/attention streams.
  - Weights arrive as one large contiguous DMA each (host pre-arranges
    partition-major layouts); x blocks in 512 KiB pieces.
"""

import sys

sys.path.insert(0, "/opt/trn_rl_repo")

import numpy as np
import ml_dtypes

import concourse.bass as bass
import concourse.tile as tile
from concourse import bacc
from concourse import mybir
from concourse.bass import ts
from concourse.bass_utils import run_bass_kernel_spmd

BF16 = ml_dtypes.bfloat16

B = 2           # batch
S = 2048        # sequence
D = 2048        # model dim
HL = 4          # heads per core (local)
DH = 128        # head dim
NC_ = 16        # d-chunks of 128
NT = 16         # seq tiles of 128
NB = 4          # seq blocks of 512
BLK = 512
EPS = 1e-6

_CACHED_NC = None


def build_nc():
    dt = mybir.dt
    nc = bacc.Bacc()

    xT = nc.declare_dram_parameter("xT", [NB, 128, NC_ * BLK], dt.bfloat16, isOutput=False)
    wq = nc.declare_dram_parameter("wq", [128, NC_ * HL * DH], dt.bfloat16, isOutput=False)
    wk = nc.declare_dram_parameter("wk", [128, NC_ * DH], dt.bfloat16, isOutput=False)
    wv = nc.declare_dram_parameter("wv", [128, NC_ * DH], dt.bfloat16, isOutput=False)
    wo = nc.declare_dram_parameter("wo", [128, HL * D], dt.bfloat16, isOutput=False)
    cosq = nc.declare_dram_parameter("cosq", [128, S], dt.bfloat16, isOutput=False)
    sinq = nc.declare_dram_parameter("sinq", [128, S], dt.bfloat16, isOutput=False)
    cosk = nc.declare_dram_parameter("cosk", [128, S], dt.bfloat16, isOutput=False)
    sink = nc.declare_dram_parameter("sink", [128, S], dt.bfloat16, isOutput=False)
    tri2 = nc.declare_dram_parameter("tri2", [128, 2, 128], dt.float32, isOutput=False)
    rt = nc.declare_dram_parameter("rt", [128, 128], dt.bfloat16, isOutput=False)
    eye = nc.declare_dram_parameter("eye", [128, 128], dt.bfloat16, isOutput=False)
    ones = nc.declare_dram_parameter("ones", [128, 1], dt.bfloat16, isOutput=False)
    out = nc.declare_dram_parameter("out", [S, D], dt.float32, isOutput=True)

    with tile.TileContext(nc) as tc:
        with (
            tc.tile_pool(name="singles", bufs=1) as singles,
            tc.tile_pool(name="xpool", bufs=2) as xpool,
            tc.tile_pool(name="work", bufs=2) as work,
            tc.tile_pool(name="expp", bufs=4) as expp,
            tc.tile_pool(name="accp", bufs=3) as accp,
            tc.tile_pool(name="rows", bufs=2) as rows,
            tc.tile_pool(name="craw", bufs=6) as craw,
            tc.tile_pool(name="outp", bufs=4) as outp,
            tc.tile_pool(name="ctxp", bufs=2) as ctxp,
            tc.tile_pool(name="pS", bufs=2, space="PSUM") as pS,
            tc.tile_pool(name="pC", bufs=2, space="PSUM") as pC,
            tc.tile_pool(name="pR", bufs=1, space="PSUM") as pR,
        ):
            # ---- resident SBUF tensors ----
            wq_s = singles.tile([128, NC_, HL * DH], dt.bfloat16)
            wk_s = singles.tile([128, NC_, DH], dt.bfloat16)
            wv_s = singles.tile([128, NC_, DH], dt.bfloat16)
            wo_s = singles.tile([128, HL, D], dt.bfloat16)
            cosq_s = singles.tile([128, S], dt.bfloat16)
            sinq_s = singles.tile([128, S], dt.bfloat16)
            cosk_s = singles.tile([128, S], dt.bfloat16)
            sink_s = singles.tile([128, S], dt.bfloat16)
            tri2_s = singles.tile([128, 2, 128], dt.float32)
            rt_s = singles.tile([128, 128], dt.bfloat16)
            eye_s = singles.tile([128, 128], dt.bfloat16)
            ones_s = singles.tile([128, 1], dt.bfloat16)
            rk_s = singles.tile([128, NT], dt.float32)
            eps_s = singles.tile([128, 1], dt.float32)
            nc.vector.memset(eps_s, EPS)
            qT_s = singles.tile([128, HL, S], dt.bfloat16)
            kT_s = singles.tile([128, S], dt.bfloat16)
            vT_s = singles.tile([128, S], dt.bfloat16)
            v_s = singles.tile([128, NT, DH], dt.bfloat16)

            # ---- input DMAs (weights/tables; xT streams per block) ----
            # gpsimd (SW-DGE) queue, one large transfer each, by first use.
            # halves keep the DMA access patterns 2D: a fully-contiguous
            # [128, N] source collapses to one descriptor whose element
            # count overflows the 16-bit src_num_elem ISA field.
            for half in range(2):
                hc = NC_ // 2
                nc.gpsimd.dma_start(
                    out=wk_s[:, half * hc : (half + 1) * hc, :],
                    in_=wk[:, half * hc * DH : (half + 1) * hc * DH],
                )
            for half in range(2):
                hc = NC_ // 2
                nc.gpsimd.dma_start(
                    out=wv_s[:, half * hc : (half + 1) * hc, :],
                    in_=wv[:, half * hc * DH : (half + 1) * hc * DH],
                )
            for quarter in range(4):
                qc = NC_ // 4
                nc.gpsimd.dma_start(
                    out=wq_s[:, quarter * qc : (quarter + 1) * qc, :],
                    in_=wq[:, quarter * qc * HL * DH : (quarter + 1) * qc * HL * DH],
                )
            nc.gpsimd.dma_start(out=eye_s[:], in_=eye[:])
            nc.gpsimd.dma_start(out=cosk_s[:], in_=cosk[:])
            nc.gpsimd.dma_start(out=sink_s[:], in_=sink[:])
            nc.gpsimd.dma_start(out=rt_s[:], in_=rt[:])
            nc.gpsimd.dma_start(out=ones_s[:], in_=ones[:])
            nc.gpsimd.dma_start(out=tri2_s[:], in_=tri2[:])
            nc.gpsimd.dma_start(out=cosq_s[:], in_=cosq[:])
            nc.gpsimd.dma_start(out=sinq_s[:], in_=sinq[:])
            for half in range(2):
                hh_ = HL // 2
                nc.gpsimd.dma_start(
                    out=wo_s[:, half * hh_ : (half + 1) * hh_, :],
                    in_=wo[:, half * hh_ * D : (half + 1) * hh_ * D],
                )

            Exp = mybir.ActivationFunctionType.Exp
            Ln = mybir.ActivationFunctionType.Ln
            Copy = mybir.ActivationFunctionType.Copy

            # Pre-load the one table set that covers exp+ln+copy; without
            # this the insertion pass alternates exp_and_others <->
            # natural_log (41 loads x ~2.7us of ACT time).
            from concourse.hw_specs import get_activation_tables

            tabs = get_activation_tables(nc.m.arch)
            set_id = list(tabs).index("natural_log_exp_and_others")
            nc.scalar.add_instruction(
                mybir.InstLoadActFuncSet(
                    name=nc.get_next_instruction_name(),
                    act_func_set_id=set_id,
                    ins=[],
                    outs=[],
                )
            )

            def load_xb(Bb):
                xb = xpool.tile([128, NC_, BLK], dt.bfloat16, tag="xblk")
                # block 0 is on the critical path: finer pieces let the
                # first K matmul start one piece earlier
                step = 2 if Bb == 0 else 4
                for g4 in range(NC_ // step):
                    nc.sync.dma_start(
                        out=xb[:, step * g4 : step * (g4 + 1), :],
                        in_=xT[Bb, :, step * g4 * BLK : step * (g4 + 1) * BLK],
                    )
                return xb

            def rope(dst_ap, qn, cos_s, sin_s, Bb):
                """dst = rope(qn) in bf16; qn is SBUF bf16 [128, 512]."""
                ps_rot = pC.tile([128, BLK], dt.float32, tag="acc")
                nc.tensor.matmul(ps_rot, rt_s, qn, start=True, stop=True)
                t1 = work.tile([128, BLK], dt.bfloat16, tag="t1")
                nc.vector.tensor_mul(t1, ps_rot, sin_s[:, ts(Bb, BLK)])
                t2 = work.tile([128, BLK], dt.bfloat16, tag="t2")
                nc.vector.tensor_mul(t2, qn, cos_s[:, ts(Bb, BLK)])
                nc.vector.tensor_add(dst_ap, t2, t1)

            # out-proj filler machinery: tiles of completed blocks are
            # interleaved into later PE streams to cover ACT-bound spans.
            filler = []

            def emit_filler(pool=None, tag="row", on_act=False):
                if not filler:
                    return
                ctxn_t, stl, st, oc = filler.pop(0)
                ps_o = (pool or pR).tile([128, BLK], dt.float32, tag=tag)
                for h in range(HL):
                    nc.tensor.matmul(
                        ps_o, ctxn_t[:, h, ts(stl, 128)],
                        wo_s[:, h, ts(oc, BLK)],
                        start=(h == 0), stop=(h == HL - 1),
                    )
                osb = outp.tile([128, BLK], dt.float32, tag="osb")
                if on_act:
                    # drain phase only: ACT has no exp stream to feed there
                    nc.scalar.activation(osb, ps_o, Copy)
                else:
                    nc.vector.tensor_copy(osb, ps_o)
                nc.sync.dma_start(out=out[ts(st, 128), ts(oc, BLK)], in_=osb)

            xb = load_xb(0)
            for Bb in range(NB):
                # ---- K + V projections, c-major ----
                ps_k = pC.tile([128, BLK], dt.float32, tag="acc")
                ps_v = pC.tile([128, BLK], dt.float32, tag="acc")
                for c in range(NC_):
                    nc.tensor.matmul(
                        ps_k, wk_s[:, c, :], xb[:, c, :],
                        start=(c == 0), stop=(c == NC_ - 1),
                    )
                    nc.tensor.matmul(
                        ps_v, wv_s[:, c, :], xb[:, c, :],
                        start=(c == 0), stop=(c == NC_ - 1),
                    )
                kraw = work.tile([128, BLK], dt.bfloat16, tag="raw")
                nc.scalar.activation(kraw, ps_k, Copy)
                nc.vector.tensor_copy(vT_s[:, ts(Bb, BLK)], ps_v)
                k2 = work.tile([128, BLK], dt.bfloat16, tag="sq")
                nc.vector.tensor_mul(k2, kraw, kraw)
                ps_rk = pR.tile([128, 4], dt.float32, tag="row")
                for tt in range(4):
                    nc.tensor.matmul(
                        ps_rk[:, tt : tt + 1], k2[:, ts(tt, 128)], ones_s,
                        start=True, stop=True,
                    )
                # rk = exp(-0.5*ln(ms/DH + eps))
                nc.scalar.activation(ps_rk, ps_rk, Ln, scale=1.0 / DH, bias=eps_s)
                nc.scalar.activation(
                    rk_s[:, Bb * 4 : Bb * 4 + 4], ps_rk, Exp, scale=-0.5
                )
                rope(kT_s[:, ts(Bb, BLK)], kraw, cosk_s, sink_s, Bb)

                # V transposes: vT [dh, sk] -> v_s [sk, t, dh]
                for tt in range(4):
                    t = Bb * 4 + tt
                    t_ps = pC.tile([128, DH], dt.bfloat16, tag="acc")
                    nc.tensor.transpose(t_ps, vT_s[:, ts(t, 128)], eye_s)
                    nc.vector.tensor_copy(v_s[:, t, :], t_ps)

                # ---- Q projections for this block, rms via ln/exp ----
                # software-pipelined: matmuls+rowsums for both pairs first,
                # then the rope chains (whose latency hides under them),
                # with out-proj filler between.
                qraws = {}
                rqs = {}
                for hp in range(2):
                    ps_ss = pR.tile([1, 2, BLK], dt.float32, tag="row")
                    for hh in range(2):
                        h = hp * 2 + hh
                        ps_q = pS.tile([128, 2, BLK], dt.float32, tag="big")
                        for c in range(NC_):
                            nc.tensor.matmul(
                                ps_q[:, 0, :], wq_s[:, c, ts(h, DH)],
                                xb[:, c, :],
                                start=(c == 0), stop=(c == NC_ - 1),
                            )
                        qraw = work.tile([128, BLK], dt.bfloat16, tag=f"qr{h}")
                        nc.scalar.activation(qraw, ps_q[:, 0, :], Copy)
                        qraws[h] = qraw
                        q2 = work.tile([128, BLK], dt.bfloat16, tag="sq")
                        nc.vector.tensor_mul(q2, qraw, qraw)
                        nc.tensor.matmul(
                            ps_ss[:, hh, :], ones_s, q2, start=True, stop=True
                        )
                    # rq = exp(-0.5*ln(ss/DH + eps)), fp32 rows
                    nc.scalar.activation(
                        ps_ss, ps_ss, Ln, scale=1.0 / DH, bias=eps_s[:1]
                    )
                    rq = rows.tile([1, 2, BLK], dt.float32, tag=f"rq{hp}")
                    nc.scalar.activation(rq, ps_ss, Exp, scale=-0.5)
                    rqs[hp] = rq
                    emit_filler(pool=pC, tag="acc")
                for hp in range(2):
                    for hh in range(2):
                        h = hp * 2 + hh
                        rq_b = work.tile([128, BLK], dt.float32, tag="rqb")
                        nc.gpsimd.partition_broadcast(rq_b, rqs[hp][:, hh, :])
                        qn = work.tile([128, BLK], dt.bfloat16, tag="qn")
                        nc.vector.tensor_mul(qn, qraws[h], rq_b)
                        rope(qT_s[:, h, ts(Bb, BLK)], qn, cosq_s, sinq_s, Bb)
                        emit_filler(pool=pR, tag="row")

                # ---- prefetch next block's x while attention runs ----
                if Bb + 1 < NB:
                    xb = load_xb(Bb + 1)

                # ---- attention for this block, head pairs ----
                ctxn_t = ctxp.tile([128, HL, BLK], dt.bfloat16, tag="ctxn")
                ntile = 4 * Bb + 4
                for hp in range(2):
                    h0 = hp * 2
                    ps_ctx = []
                    for _ in range(2):
                        ps_cx = pC.tile([128, BLK], dt.float32, tag="acc")
                        ps_ctx.append(ps_cx)
                    acc = accp.tile([128, 2, BLK], dt.bfloat16, tag="sumacc")
                    expS_last = None
                    pending = None
                    for t in range(ntile):
                        j0 = max(0, t * 128 - Bb * BLK)
                        ps_S = pS.tile([128, 2, BLK], dt.float32, tag="big")
                        for hh in range(2):
                            nc.tensor.matmul(
                                ps_S[:, hh, j0:],
                                kT_s[:, ts(t, 128)],
                                qT_s[:, h0 + hh, Bb * BLK + j0 : (Bb + 1) * BLK],
                                start=True, stop=True,
                            )
                        if t * 128 >= Bb * BLK:  # diagonal tile: causal mask
                            nc.vector.tensor_add(
                                ps_S[:, :, j0 : j0 + 128],
                                ps_S[:, :, j0 : j0 + 128],
                                tri2_s,
                            )
                        expS = expp.tile([128, 2, BLK], dt.bfloat16, tag="exp")
                        nc.scalar.activation(
                            expS[:, :, j0:], ps_S[:, :, j0:], Exp,
                            scale=rk_s[:, t : t + 1],
                        )
                        if t < 2:
                            # pair start: the first ctx matmuls wait on
                            # exp(t0) and the ctx-bank release (DVE craw
                            # backlog); a filler emitted BEFORE them fills
                            # the PE FIFO head-of-line block
                            emit_filler(pool=pR, tag="row")
                        for hh in range(2):
                            nc.tensor.matmul(
                                ps_ctx[hh][:, j0:], v_s[:, t, :],
                                expS[:, hh, j0:],
                                start=(t == 0), stop=(t == ntile - 1),
                            )
                        # accumulate exp sums on DVE, deferred one iteration
                        # so the next diag tile's tri-add doesn't queue
                        # behind this add in the DVE FIFO (which would delay
                        # exp and hold the score slot); the last tile is
                        # folded into the denominator matmul instead.
                        if pending is not None:
                            eP, jP, isF = pending
                            if isF:
                                nc.vector.tensor_copy(acc, eP)
                            else:
                                nc.vector.tensor_add(
                                    acc[:, :, jP:], acc[:, :, jP:],
                                    eP[:, :, jP:],
                                )
                        if t == ntile - 1:
                            expS_last = (expS, j0)
                            pending = None
                        else:
                            pending = (expS, j0, t == 0)
                        if t >= 2:
                            emit_filler(pool=pR, tag="row")
                    # pair denominators: d = ones @ acc + ones @ expS_last
                    eL, jL = expS_last
                    ps_row = pR.tile([1, 2, BLK], dt.float32, tag="row")
                    for hh in range(2):
                        nc.tensor.matmul(
                            ps_row[:, hh, :], ones_s, acc[:, hh, :],
                            start=True, stop=False,
                        )
                        nc.tensor.matmul(
                            ps_row[:, hh, jL:], ones_s, eL[:, hh, jL:],
                            start=False, stop=True,
                        )
                    nc.scalar.activation(ps_row, ps_row, Ln)
                    recip = rows.tile([1, 2, BLK], dt.bfloat16, tag="recip")
                    nc.scalar.activation(recip, ps_row, Exp, scale=-1.0)
                    recip_b = craw.tile([128, 2, BLK], dt.bfloat16, tag="rb")
                    nc.gpsimd.partition_broadcast(recip_b, recip[:])
                    for hh in range(2):
                        craw_t = craw.tile([128, BLK], dt.bfloat16, tag="craw")
                        nc.vector.tensor_copy(craw_t, ps_ctx[hh])
                        nc.vector.tensor_mul(
                            ctxn_t[:, h0 + hh, :], craw_t, recip_b[:, hh, :]
                        )
                # queue this block's out-proj tiles as filler
                for stl in range(4):
                    st = Bb * 4 + stl
                    for oc in range(4):
                        filler.append((ctxn_t, stl, st, oc))

            # drain remaining out-proj tiles (last block): rotate across
            # the now-free PSUM pools so the groups pipeline.
            i = 0
            while filler:
                pool, tag = [(pS, "big"), (pC, "acc"), (pR, "row")][i % 3]
                emit_filler(pool=pool, tag=tag, on_act=(i % 2 == 0))
                i += 1
    nc.finalize()
    return nc


def _host_inputs(x, cos, sin, Wq, Wk, Wv, Wo, qn_w, kn_w):
    """Build the 8 per-core input maps (host-side sharding + layout prep)."""
    scale = DH ** -0.5
    qn_rot = np.concatenate([qn_w[64:], qn_w[:64]])
    kn_rot = np.concatenate([kn_w[64:], kn_w[:64]])
    cosq = (cos.T * qn_w[:, None] * scale).astype(BF16)
    sinq = (sin.T * qn_rot[:, None] * scale).astype(BF16)
    cosk = (cos.T * kn_w[:, None]).astype(BF16)
    sink = (sin.T * kn_rot[:, None]).astype(BF16)
    ii = np.arange(128)
    tri = np.where(ii[None, :] < ii[:, None], -1e30, 0.0).astype(np.float32)
    tri2 = np.ascontiguousarray(np.stack([tri, tri], axis=1))  # [128, 2, 128]
    R = np.zeros((128, 128), dtype=np.float32)
    R[np.arange(64), np.arange(64) + 64] = -1.0
    R[np.arange(64, 128), np.arange(64)] = 1.0
    rt = R.T.astype(BF16)
    eye = np.eye(128, dtype=BF16)
    ones = np.ones((128, 1), dtype=BF16)

    def pmajor(w, ncol):
        # [NC_*128, ncol] -> [128, NC_*ncol] with w[c*128+p, j] at [p, c*ncol+j]
        n = w.shape[0] // 128
        return np.ascontiguousarray(
            w.reshape(n, 128, ncol).transpose(1, 0, 2).reshape(128, n * ncol)
        ).astype(BF16)

    in_maps = []
    for core in range(8):
        b, g = core // 4, core % 4
        # [NB, 128, NC_*BLK]: xT[Bb, dc, c*BLK+sb] = x[b][Bb*512+sb, c*128+dc]
        xTb = np.ascontiguousarray(
            x[b].reshape(NB, BLK, NC_, 128).transpose(0, 3, 2, 1).reshape(NB, 128, NC_ * BLK)
        ).astype(BF16)
        in_maps.append({
            "xT": xTb,
            "wq": pmajor(Wq[:, g * 512 : (g + 1) * 512], 512),
            "wk": pmajor(Wk[:, g * 128 : (g + 1) * 128], 128),
            "wv": pmajor(Wv[:, g * 128 : (g + 1) * 128], 128),
            "wo": pmajor(Wo[g * 512 : (g + 1) * 512, :], D),
            "cosq": cosq, "sinq": sinq, "cosk": cosk, "sink": sink,
            "tri2": tri2, "rt": rt, "eye": eye, "ones": ones,
        })
    return in_maps


def kernel(x, mask, cos, sin, Wq, Wk, Wv, Wo, qn_w, kn_w, _trace=False):
    global _CACHED_NC
    x = np.asarray(x, dtype=np.float32)
    cos = np.asarray(cos, dtype=np.float32)
    sin = np.asarray(sin, dtype=np.float32)
    Wq = np.asarray(Wq, dtype=np.float32)
    Wk = np.asarray(Wk, dtype=np.float32)
    Wv = np.asarray(Wv, dtype=np.float32)
    Wo = np.asarray(Wo, dtype=np.float32)
    qn_w = np.asarray(qn_w, dtype=np.float32)
    kn_w = np.asarray(kn_w, dtype=np.float32)

    if _CACHED_NC is None:
        _CACHED_NC = build_nc()
    nc = _CACHED_NC
    in_maps = _host_inputs(x, cos, sin, Wq, Wk, Wv, Wo, qn_w, kn_w)
    res = run_bass_kernel_spmd(nc, in_maps, list(range(8)), trace=_trace)
    out = np.zeros((B, S, D), dtype=np.float32)
    for core in range(8):
        b = core // 4
        out[b] += np.asarray(res.results[core]["out"], dtype=np.float32)
    if _trace:
        return out, res
    return out
